# revision 1
# baseline (speedup 1.0000x reference)
"""Trainium2 Bass kernel for nn_Conv2d_20590073217670.

Conv2d: input [32,64,64,64] (NCHW), weight [576,128] (unfold layout:
row = ci*9 + a*3 + b for tap (a,b)), bias [1,128,1,1], stride 1, pad 1.
Output [32,128,64,64].

Strategy: data-parallel over batch — 4 images per NeuronCore, 8 cores.
Per image, implicit GEMM: out[co, y, x] = sum_{a,b,ci} W[ci,a,b,co] *
img[ci, y+a-1, x+b-1].  SBUF holds the image on partitions 0:64 and a
one-row-down shifted copy on partitions 64:128, so a single K=128
matmul accumulates two vertical taps (a, a+1) at once.  The rounded
fp32r image is stored column-padded ([128, 64, 66], zero borders), so
every matmul is a full 64-wide slide satisfying the fp32r ISA
restrictions (even innermost count, 8B-aligned full-bank PSUM output).
Row borders are handled by restricting output rows (PSUM has_written
zero-fill keeps partial accumulation exact).  DVE produces all matmul
inputs (fp32->fp32r rounding) and evicts PSUM with a fused bias add.
"""
import sys

for _p in ("/opt/trn_rl_repo", "/root/.axon_site/_ro/trn_rl_repo"):
    if _p not in sys.path:
        sys.path.append(_p)

import numpy as np
from contextlib import ExitStack

import concourse.bacc as bacc
import concourse.tile as tile
from concourse import mybir
from concourse.bass_utils import run_bass_kernel_spmd

f32 = mybir.dt.float32
f32r = mybir.dt.float32r

N_CORES = 8
NB = 4  # images per core


def build_nc():
    nc = bacc.Bacc()
    x = nc.declare_dram_parameter("x", [NB, 64, 64, 64], f32, isOutput=False)
    w = nc.declare_dram_parameter("w", [576, 128], f32, isOutput=False)
    bias = nc.declare_dram_parameter("b", [128, 1], f32, isOutput=False)
    out = nc.declare_dram_parameter("out", [NB, 128, 64, 64], f32, isOutput=True)

    with tile.TileContext(nc) as tc, ExitStack() as ctx:
        const = ctx.enter_context(tc.tile_pool(name="const", bufs=1))
        xs_pool = ctx.enter_context(tc.tile_pool(name="xs", bufs=3))
        xr_pool = ctx.enter_context(tc.tile_pool(name="xr", bufs=3))
        ob_pool = ctx.enter_context(tc.tile_pool(name="ob", bufs=2))
        ps_pool = ctx.enter_context(tc.tile_pool(name="ps", bufs=8, space="PSUM"))

        # ---- weights: one [128, 9, 128] tile; partition p<64 holds channel
        # p's taps 0..8, partition 64+ci holds channel ci's taps 3..8 at
        # slots 0..5 (tap axis pre-shifted by -3).  Then the lhsT view
        # wr[:, t, :] pairs taps (t, t+3) across the partition halves:
        #   t in 0..2  -> taps (0,b) & (1,b)
        #   t in 3..5  -> taps (1,b) & (2,b)
        w3 = w[:].rearrange("(c t) m -> c t m", t=9)
        ws = const.tile([128, 9, 128], f32)
        wr = const.tile([128, 9, 128], f32r)
        bt = const.tile([128, 1], f32)
        zc = const.tile([128, 64, 1], f32)
        nc.sync.dma_start(out=ws[0:64, :, :], in_=w3)
        nc.sync.dma_start(out=ws[64:128, 0:6, :], in_=w3[:, 3:9, :])
        nc.sync.dma_start(out=bt[:], in_=bias[:])
        nc.vector.memset(zc[:], 0.0)
        nc.vector.tensor_copy(wr[0:64, :, :], ws[0:64, :, :])
        nc.vector.tensor_copy(wr[64:128, 0:6, :], ws[64:128, 0:6, :])

        for n in range(NB):
            xs = xs_pool.tile([128, 64, 64], f32)
            xr = xr_pool.tile([128, 64, 66], f32r)
            # image rows on partitions 0:64; one-row-down copy on 64:128
            nc.sync.dma_start(out=xs[0:64, :, :], in_=x[n])
            nc.sync.dma_start(out=xs[64:128, 0:63, :], in_=xs[0:64, 1:64, :])
            # fp32 -> fp32r rounding (DVE) into the column-padded layout;
            # upper-half row 63 is never read.  Zero border columns.
            nc.vector.tensor_copy(xr[:, 0:63, 1:65], xs[:, 0:63, :])
            nc.vector.tensor_copy(xr[0:64, 63, 1:65], xs[0:64, 63, :])
            nc.vector.tensor_copy(xr[:, :, 0:1], zc[:])
            nc.vector.tensor_copy(xr[:, :, 65:66], zc[:])

            osb = ob_pool.tile([128, 64, 64], f32)
            for blk in range(8):
                y0 = blk * 8
                P = ps_pool.tile([128, 8, 64], f32)
                if blk == 0:
                    pair_t, pr0 = 3, 0      # taps (1,2), rhs rows y0..y0+7
                else:
                    pair_t, pr0 = 0, y0 - 1  # taps (0,1), rhs rows y0-1..y0+6
                # b=1 first: full [8,64] coverage zero-fills the whole bank
                for k, b in enumerate((1, 0, 2)):
                    nc.tensor.matmul(
                        P[:, 0:8, :],
                        wr[:, pair_t + b, :],
                        xr[:, pr0:pr0 + 8, b:b + 64],
                        start=(k == 0), stop=False,
                    )
                # remaining vertical tap as K=64 single on partitions 0:64
                for k, b in enumerate((1, 0, 2)):
                    last = k == 2
                    if blk == 0:
                        # tap (0,b): out rows 1..7 read img rows 0..6
                        nc.tensor.matmul(
                            P[:, 1:8, :], wr[0:64, b, :],
                            xr[0:64, 0:7, b:b + 64],
                            start=False, stop=last,
                        )
                    elif blk == 7:
                        # tap (2,b): out rows 56..62 read img rows 57..63
                        nc.tensor.matmul(
                            P[:, 0:7, :], wr[0:64, 6 + b, :],
                            xr[0:64, 57:64, b:b + 64],
                            start=False, stop=last,
                        )
                    else:
                        nc.tensor.matmul(
                            P[:, 0:8, :], wr[0:64, 6 + b, :],
                            xr[0:64, y0 + 1:y0 + 9, b:b + 64],
                            start=False, stop=last,
                        )
                nc.vector.tensor_scalar_add(osb[:, y0:y0 + 8, :], P[:, :, :], bt[:])

            nc.sync.dma_start(out=out[n], in_=osb[:])

    nc.finalize()
    return nc


_NC = None


def _get_nc():
    global _NC
    if _NC is None:
        _NC = build_nc()
    return _NC


def kernel(**inputs) -> np.ndarray:
    x = np.ascontiguousarray(np.asarray(inputs["input"], dtype=np.float32))
    w = np.ascontiguousarray(np.asarray(inputs["weight"], dtype=np.float32))
    b = np.ascontiguousarray(
        np.asarray(inputs["bias"], dtype=np.float32).reshape(128, 1))
    nc = _get_nc()
    in_maps = [
        {"x": x[c * NB:(c + 1) * NB], "w": w, "b": b} for c in range(N_CORES)
    ]
    res = run_bass_kernel_spmd(nc, in_maps, list(range(N_CORES)))
    return np.concatenate([r["out"] for r in res.results], axis=0)



# revision 2
# speedup vs baseline: 1.2680x; 1.2680x over previous
"""Trainium2 Bass kernel for nn_Conv2d_20590073217670.

Conv2d: input [32,64,64,64] (NCHW), weight [576,128] (unfold layout:
row = ci*9 + a*3 + b for tap (a,b)), bias [1,128,1,1], stride 1, pad 1.
Output [32,128,64,64].

Strategy: data-parallel over batch — 4 images per NeuronCore, 8 cores.

Per image, implicit GEMM in bf16 (PSUM accumulates fp32; rel err ~3e-3
vs the 2e-2 gate).  The host pre-pads each image to [64, 66, 66] bf16
with zero borders (rows 0/65, cols 0/65), so the device does ZERO
compute for data prep: one HBM DMA fills partitions 0:64 with the
padded image P (P[r] = img row r-1), a second HBM DMA fills partitions
64:128 with the one-row-down copy U (U[r] = P[r+1] = img row r).

With that layout every 8-row output block is uniform:
  out[co,y,x] = sum_{a,b,ci} W[ci,a,b,co] * img[ci, y+a-1, x+b-1]
  - taps (0,b)+(1,b): one K=128 matmul, lhsT = [W[:,0,b,:]; W[:,1,b,:]],
    rhs = xim[:, y0:y0+8, b:b+64]          (3 matmuls)
  - tap (2,b): one K=64 matmul on partitions 64:128 (U),
    rhs = xim[64:128, y0+1:y0+9, b:b+64]   (3 matmuls)
No border special-casing: zero rows/cols make out-of-image taps exact.

bf16 streams the moving operand at 1 col/cycle (vs ~2+ for fp32r), so
the 192 N=512 matmuls take ~42us.  PSUM eviction (fused bias add) is
split between ScalarE (activation Identity + bias) and VectorE
(tensor_scalar_add) on alternating banks so neither blocks the PE.
"""
import sys

for _p in ("/opt/trn_rl_repo", "/root/.axon_site/_ro/trn_rl_repo"):
    if _p not in sys.path:
        sys.path.append(_p)

import numpy as np
import ml_dtypes
from contextlib import ExitStack

import concourse.bacc as bacc
import concourse.tile as tile
from concourse import mybir
from concourse.bass_utils import run_bass_kernel_spmd

f32 = mybir.dt.float32
bf16 = mybir.dt.bfloat16

N_CORES = 8
NB = 4  # images per core


def build_nc():
    nc = bacc.Bacc()
    # host-padded bf16 image: [NB, 64ch, 66, 66], zero borders
    xp = nc.declare_dram_parameter("x", [NB, 64, 66, 66], bf16, isOutput=False)
    # host-packed weights: [128, 6, 128] bf16
    #   w[:, b, :]        = [W[:,0,b,:]; W[:,1,b,:]]  (tap pair, K=128)
    #   w[64:128, 3+b, :] = W[:,2,b,:]                (tap 2 single, K=64)
    w = nc.declare_dram_parameter("w", [128, 6, 128], bf16, isOutput=False)
    bias = nc.declare_dram_parameter("b", [128, 1], f32, isOutput=False)
    out = nc.declare_dram_parameter("out", [NB, 128, 64, 64], f32, isOutput=True)

    with tile.TileContext(nc) as tc, ExitStack() as ctx:
        const = ctx.enter_context(tc.tile_pool(name="const", bufs=1))
        xim_pool = ctx.enter_context(tc.tile_pool(name="xim", bufs=3))
        ob_pool = ctx.enter_context(tc.tile_pool(name="ob", bufs=2))
        ps_pool = ctx.enter_context(tc.tile_pool(name="ps", bufs=8, space="PSUM"))

        wt = const.tile([128, 6, 128], bf16)
        bt = const.tile([128, 1], f32)
        nc.sync.dma_start(out=wt[:], in_=w[:])
        nc.sync.dma_start(out=bt[:], in_=bias[:])

        for n in range(NB):
            xim = xim_pool.tile([128, 66, 66], bf16)
            # P on partitions 0:64, U (one row down) on 64:128.
            nc.sync.dma_start(out=xim[0:64, :, :], in_=xp[n])
            nc.sync.dma_start(out=xim[64:128, 0:65, :], in_=xp[n, :, 1:66, :])

            osb = ob_pool.tile([128, 64, 64], f32)
            for blk in range(8):
                y0 = blk * 8
                P = ps_pool.tile([128, 8, 64], f32)
                for k, b in enumerate((0, 1, 2)):
                    nc.tensor.matmul(
                        P[:, :, :],
                        wt[:, b, :],
                        xim[:, y0:y0 + 8, b:b + 64],
                        start=(k == 0), stop=False,
                    )
                for k, b in enumerate((0, 1, 2)):
                    nc.tensor.matmul(
                        P[:, :, :],
                        wt[64:128, 3 + b, :],
                        xim[64:128, y0 + 1:y0 + 9, b:b + 64],
                        start=False, stop=(k == 2),
                    )
                # PSUM evict + bias, alternating engines per bank
                if blk % 2 == 0:
                    nc.scalar.activation(
                        osb[:, y0:y0 + 8, :], P[:, :, :],
                        mybir.ActivationFunctionType.Identity,
                        bias=bt[:, :],
                    )
                else:
                    nc.vector.tensor_scalar_add(
                        osb[:, y0:y0 + 8, :], P[:, :, :], bt[:, :])

            nc.sync.dma_start(out=out[n], in_=osb[:])

    nc.finalize()
    return nc


_NC = None


def _get_nc():
    global _NC
    if _NC is None:
        _NC = build_nc()
    return _NC


def _prep(inputs):
    x = np.asarray(inputs["input"], dtype=np.float32)
    w = np.asarray(inputs["weight"], dtype=np.float32)
    b = np.asarray(inputs["bias"], dtype=np.float32).reshape(128, 1)

    # pad to [32, 64, 66, 66] and cast bf16
    xp = np.zeros((x.shape[0], 64, 66, 66), dtype=ml_dtypes.bfloat16)
    xp[:, :, 1:65, 1:65] = x

    wr = w.reshape(64, 3, 3, 128)  # [ci, a, b, co]
    wa = np.zeros((128, 6, 128), dtype=ml_dtypes.bfloat16)
    for bb in range(3):
        wa[0:64, bb, :] = wr[:, 0, bb, :]
        wa[64:128, bb, :] = wr[:, 1, bb, :]
        wa[64:128, 3 + bb, :] = wr[:, 2, bb, :]
    return xp, wa, np.ascontiguousarray(b)


def kernel(**inputs) -> np.ndarray:
    xp, wa, b = _prep(inputs)
    nc = _get_nc()
    in_maps = [
        {"x": xp[c * NB:(c + 1) * NB], "w": wa, "b": b} for c in range(N_CORES)
    ]
    res = run_bass_kernel_spmd(nc, in_maps, list(range(N_CORES)))
    return np.concatenate([r["out"] for r in res.results], axis=0)


# revision 3
# speedup vs baseline: 2.0495x; 1.6162x over previous
"""Trainium2 Bass kernel for nn_Conv2d_20590073217670.

Conv2d: input [32,64,64,64] (NCHW), weight [576,128] (unfold layout:
row = ci*9 + a*3 + b for tap (a,b)), bias [1,128,1,1], stride 1, pad 1.
Output [32,128,64,64].

Strategy: data-parallel over batch — 4 images per NeuronCore, 8 cores.

Per image, implicit GEMM in bf16 (PSUM accumulates fp32; rel err ~3e-3
vs the 2e-2 gate).  The host pre-pads each image to [64, 66, 66] bf16
with zero borders (rows 0/65, cols 0/65), so the device does ZERO
compute for data prep: one HBM DMA fills partitions 0:64 with the
padded image P (P[r] = img row r-1), a second HBM DMA fills partitions
64:128 with the one-row-down copy U (U[r] = P[r+1] = img row r).

With that layout every 8-row output block is uniform:
  out[co,y,x] = sum_{a,b,ci} W[ci,a,b,co] * img[ci, y+a-1, x+b-1]
  - taps (0,b)+(1,b): one K=128 matmul, lhsT = [W[:,0,b,:]; W[:,1,b,:]],
    rhs = xim[:, y0:y0+8, b:b+64]          (3 matmuls)
  - tap (2,b): one K=64 matmul on partitions 64:128 (U),
    rhs = xim[64:128, y0+1:y0+9, b:b+64]   (3 matmuls)
No border special-casing: zero rows/cols make out-of-image taps exact.

bf16 streams the moving operand at 1 col/cycle (vs ~2+ for fp32r), so
the 192 N=512 matmuls take ~42us.  PSUM eviction (fused bias add) is
split between ScalarE (activation Identity + bias) and VectorE
(tensor_scalar_add) on alternating banks so neither blocks the PE.
"""
import sys

for _p in ("/opt/trn_rl_repo", "/root/.axon_site/_ro/trn_rl_repo"):
    if _p not in sys.path:
        sys.path.append(_p)

import numpy as np
import ml_dtypes
from contextlib import ExitStack

import concourse.bacc as bacc
import concourse.tile as tile
from concourse import mybir
from concourse.bass_utils import run_bass_kernel_spmd

f32 = mybir.dt.float32
bf16 = mybir.dt.bfloat16

N_CORES = 8
NB = 4  # images per core


def build_nc():
    nc = bacc.Bacc()
    # host-padded bf16 image: [NB, 64ch, 66, 66], zero borders
    xp = nc.declare_dram_parameter("x", [NB, 64, 66, 66], bf16, isOutput=False)
    # host-packed weights: [128, 6, 128] bf16
    #   w[:, b, :]        = [W[:,0,b,:]; W[:,1,b,:]]  (tap pair, K=128)
    #   w[64:128, 3+b, :] = W[:,2,b,:]                (tap 2 single, K=64)
    w = nc.declare_dram_parameter("w", [128, 6, 128], bf16, isOutput=False)
    bias = nc.declare_dram_parameter("b", [128, 1], f32, isOutput=False)
    out = nc.declare_dram_parameter("out", [NB, 128, 64, 64], f32, isOutput=True)

    with tile.TileContext(nc) as tc, ExitStack() as ctx:
        const = ctx.enter_context(tc.tile_pool(name="const", bufs=1))
        xim_pool = ctx.enter_context(tc.tile_pool(name="xim", bufs=3))
        ob_pool = ctx.enter_context(tc.tile_pool(name="ob", bufs=2))
        ps_pool = ctx.enter_context(tc.tile_pool(name="ps", bufs=8, space="PSUM"))

        wt = const.tile([128, 6, 128], bf16)
        bt = const.tile([128, 1], f32)
        nc.sync.dma_start(out=wt[:], in_=w[:])
        nc.sync.dma_start(out=bt[:], in_=bias[:])

        for n in range(NB):
            xim = xim_pool.tile([128, 66, 66], bf16)
            # P on partitions 0:64, U (one row down) on 64:128.
            nc.sync.dma_start(out=xim[0:64, :, :], in_=xp[n])
            nc.sync.dma_start(out=xim[64:128, 0:65, :], in_=xp[n, :, 1:66, :])

            osb = ob_pool.tile([128, 64, 64], f32)
            for blk in range(8):
                y0 = blk * 8
                P = ps_pool.tile([128, 8, 64], f32)
                for k, b in enumerate((0, 1, 2)):
                    nc.tensor.matmul(
                        P[:, :, :],
                        wt[:, b, :],
                        xim[:, y0:y0 + 8, b:b + 64],
                        start=(k == 0), stop=False,
                    )
                # tap-2 "singles" padded to K=128 (zero weights on 0:64):
                # a K=64 matmul leaves half the PE array idle, which keeps
                # the HAM activity monitor from un-throttling the PE clock
                # (measured: K=64 in the stream pins issue at 1.2 GHz).
                for k, b in enumerate((0, 1, 2)):
                    nc.tensor.matmul(
                        P[:, :, :],
                        wt[:, 3 + b, :],
                        xim[:, y0 + 1:y0 + 9, b:b + 64],
                        start=False, stop=(k == 2),
                    )
                # PSUM evict + bias, alternating engines per bank
                if blk % 2 == 0:
                    nc.scalar.activation(
                        osb[:, y0:y0 + 8, :], P[:, :, :],
                        mybir.ActivationFunctionType.Identity,
                        bias=bt[:, :],
                    )
                else:
                    nc.vector.tensor_scalar_add(
                        osb[:, y0:y0 + 8, :], P[:, :, :], bt[:, :])

            nc.sync.dma_start(out=out[n], in_=osb[:])

    nc.finalize()
    return nc


_NC = None


def _get_nc():
    global _NC
    if _NC is None:
        _NC = build_nc()
    return _NC


def _prep(inputs):
    x = np.asarray(inputs["input"], dtype=np.float32)
    w = np.asarray(inputs["weight"], dtype=np.float32)
    b = np.asarray(inputs["bias"], dtype=np.float32).reshape(128, 1)

    # pad to [32, 64, 66, 66] and cast bf16
    xp = np.zeros((x.shape[0], 64, 66, 66), dtype=ml_dtypes.bfloat16)
    xp[:, :, 1:65, 1:65] = x

    wr = w.reshape(64, 3, 3, 128)  # [ci, a, b, co]
    wa = np.zeros((128, 6, 128), dtype=ml_dtypes.bfloat16)
    for bb in range(3):
        wa[0:64, bb, :] = wr[:, 0, bb, :]
        wa[64:128, bb, :] = wr[:, 1, bb, :]
        wa[64:128, 3 + bb, :] = wr[:, 2, bb, :]
    return xp, wa, np.ascontiguousarray(b)


def kernel(**inputs) -> np.ndarray:
    xp, wa, b = _prep(inputs)
    nc = _get_nc()
    in_maps = [
        {"x": xp[c * NB:(c + 1) * NB], "w": wa, "b": b} for c in range(N_CORES)
    ]
    res = run_bass_kernel_spmd(nc, in_maps, list(range(N_CORES)))
    return np.concatenate([r["out"] for r in res.results], axis=0)


# revision 5
# speedup vs baseline: 2.2333x; 1.0897x over previous
"""Trainium2 Bass kernel for nn_Conv2d_20590073217670.

Conv2d: input [32,64,64,64] (NCHW), weight [576,128] (unfold layout:
row = ci*9 + a*3 + b for tap (a,b)), bias [1,128,1,1], stride 1, pad 1.
Output [32,128,64,64].

Strategy: data-parallel over batch — 4 images per NeuronCore, 8 cores.

Per image, implicit GEMM in bf16 (PSUM accumulates fp32; rel err ~2e-3
vs the 2e-2 gate).  The host pre-builds a [128, 66, 66] bf16 tile per
image with zero borders: partitions 0:64 hold the padded image P
(P[r] = img row r-1), partitions 64:128 hold the one-row-down copy U
(U[r] = img row r).  One contiguous HBM DMA per image, zero on-device
data prep.

Every 8-row output block is then uniform (no border special cases):
  out[co,y,x] = sum_{a,b,ci} W[ci,a,b,co] * img[ci, y+a-1, x+b-1]
  - taps (0,b)+(1,b): K=128 matmul, lhsT = [W[:,0,b,:]; W[:,1,b,:]],
    rhs = xim[:, y0:y0+8, b:b+64]           (3 matmuls)
  - tap (2,b): K=128 matmul with ZERO weights on partitions 0:64 and
    W[:,2,b,:] on 64:128, rhs = xim[:, y0+1:y0+9, b:b+64]  (3 matmuls)
    (K=64 matmuls would leave half the PE array idle, which keeps the
    HAM activity monitor from un-throttling the PE clock to 2.4 GHz —
    measured: any K=64 in the stream pins issue at 1.2 GHz.)

~26 dummy matmuls on a scratch SBUF tile run while the first image's
DMA is in flight, so the PE is already warm (K=8/8) when real work
starts.  PSUM eviction (fused bias add) alternates between ScalarE
(activation Identity + bias) and VectorE (tensor_scalar_add) so
neither engine gates the PE.  Output DMAs go in 32-row halves, issued
from ScalarE's hardware DGE so the Sync engine's serialized issue
stream only carries input DMAs.
"""
import sys

for _p in ("/opt/trn_rl_repo", "/root/.axon_site/_ro/trn_rl_repo"):
    if _p not in sys.path:
        sys.path.append(_p)

import numpy as np
import ml_dtypes
from contextlib import ExitStack

import concourse.bacc as bacc
import concourse.tile as tile
from concourse import mybir
from concourse.bass_utils import run_bass_kernel_spmd

f32 = mybir.dt.float32
bf16 = mybir.dt.bfloat16

N_CORES = 8
NB = 4  # images per core
N_WARM = 26  # dummy matmuls to warm the PE during the input DMA window


def build_nc():
    nc = bacc.Bacc()
    # host-built per-image tile: [P(66x66, zero borders) ; U(P shifted
    # one row up, row 65 = 0)], bf16
    xp = nc.declare_dram_parameter("x", [NB, 128, 66, 66], bf16, isOutput=False)
    # host-packed weights: [128, 6, 128] bf16
    #   w[:, b, :]   = [W[:,0,b,:]; W[:,1,b,:]]  (tap pair a=0,1)
    #   w[:, 3+b, :] = [0         ; W[:,2,b,:]]  (tap 2, zero-padded K=128)
    w = nc.declare_dram_parameter("w", [128, 6, 128], bf16, isOutput=False)
    bias = nc.declare_dram_parameter("b", [128, 1], f32, isOutput=False)
    out = nc.declare_dram_parameter("out", [NB, 128, 64, 64], f32, isOutput=True)

    with tile.TileContext(nc) as tc, ExitStack() as ctx:
        const = ctx.enter_context(tc.tile_pool(name="const", bufs=1))
        xim_pool = ctx.enter_context(tc.tile_pool(name="xim", bufs=3))
        ob_pool = ctx.enter_context(tc.tile_pool(name="ob", bufs=2))
        ps_pool = ctx.enter_context(tc.tile_pool(name="ps", bufs=7, space="PSUM"))
        pw_pool = ctx.enter_context(tc.tile_pool(name="pw", bufs=1, space="PSUM"))

        wt = const.tile([128, 6, 128], bf16)
        bt = const.tile([128, 1], f32)
        dummy = const.tile([128, 640], bf16)

        xims = []
        for n in range(NB):
            xim = xim_pool.tile([128, 66, 66], bf16)
            nc.sync.dma_start(out=xim[:, :, :], in_=xp[n])
            xims.append(xim)
            if n == 0:
                nc.sync.dma_start(out=wt[:], in_=w[:])
                nc.sync.dma_start(out=bt[:], in_=bias[:])

        # PE warm-up: independent matmuls into a scratch PSUM bank while
        # the input DMAs stream.  ~3.4us of sustained PE activity flips
        # the HAM clock gate to 2.4 GHz before the real stream begins.
        nc.gpsimd.memset(dummy[:], 0.0)
        Pw = pw_pool.tile([128, 8, 64], f32)
        for i in range(N_WARM):
            nc.tensor.matmul(
                Pw[:, :, :], dummy[:, 0:128],
                dummy[:, 128:640].rearrange("p (a b) -> p a b", a=8),
                start=True, stop=True,
            )

        for n in range(NB):
            xim = xims[n]
            osb = ob_pool.tile([128, 64, 64], f32)
            for blk in range(8):
                y0 = blk * 8
                P = ps_pool.tile([128, 8, 64], f32)
                for k, b in enumerate((0, 1, 2)):
                    nc.tensor.matmul(
                        P[:, :, :],
                        wt[:, b, :],
                        xim[:, y0:y0 + 8, b:b + 64],
                        start=(k == 0), stop=False,
                    )
                for k, b in enumerate((0, 1, 2)):
                    nc.tensor.matmul(
                        P[:, :, :],
                        wt[:, 3 + b, :],
                        xim[:, y0 + 1:y0 + 9, b:b + 64],
                        start=False, stop=(k == 2),
                    )
                # PSUM evict + bias, alternating engines per bank
                if blk % 2 == 0:
                    nc.scalar.activation(
                        osb[:, y0:y0 + 8, :], P[:, :, :],
                        mybir.ActivationFunctionType.Identity,
                        bias=bt[:, :],
                    )
                else:
                    nc.vector.tensor_scalar_add(
                        osb[:, y0:y0 + 8, :], P[:, :, :], bt[:, :])
                # drain the output in 32-row halves so the final store
                # overlaps compute; issue from ScalarE's DGE
                if blk == 3:
                    nc.scalar.dma_start(out=out[n, :, 0:32, :],
                                        in_=osb[:, 0:32, :])
                elif blk == 7:
                    nc.scalar.dma_start(out=out[n, :, 32:64, :],
                                        in_=osb[:, 32:64, :])

    nc.finalize()
    return nc


_NC = None


def _get_nc():
    global _NC
    if _NC is None:
        _NC = build_nc()
    return _NC


def _prep(inputs):
    x = np.asarray(inputs["input"], dtype=np.float32)
    w = np.asarray(inputs["weight"], dtype=np.float32)
    b = np.asarray(inputs["bias"], dtype=np.float32).reshape(128, 1)

    nimg = x.shape[0]
    xb = x.astype(ml_dtypes.bfloat16)
    xf = np.zeros((nimg, 128, 66, 66), dtype=ml_dtypes.bfloat16)
    xf[:, 0:64, 1:65, 1:65] = xb          # P: rows 1..64 = img rows 0..63
    xf[:, 64:128, 0:64, 1:65] = xb        # U: rows 0..63 = img rows 0..63

    wr = w.reshape(64, 3, 3, 128)  # [ci, a, b, co]
    wa = np.zeros((128, 6, 128), dtype=ml_dtypes.bfloat16)
    for bb in range(3):
        wa[0:64, bb, :] = wr[:, 0, bb, :]
        wa[64:128, bb, :] = wr[:, 1, bb, :]
        wa[64:128, 3 + bb, :] = wr[:, 2, bb, :]
    return xf, wa, np.ascontiguousarray(b)


def kernel(**inputs) -> np.ndarray:
    xf, wa, b = _prep(inputs)
    nc = _get_nc()
    in_maps = [
        {"x": xf[c * NB:(c + 1) * NB], "w": wa, "b": b} for c in range(N_CORES)
    ]
    res = run_bass_kernel_spmd(nc, in_maps, list(range(N_CORES)))
    return np.concatenate([r["out"] for r in res.results], axis=0)


# revision 6
# speedup vs baseline: 2.2799x; 1.0209x over previous
"""Trainium2 Bass kernel for nn_Conv2d_20590073217670.

Conv2d: input [32,64,64,64] (NCHW), weight [576,128] (unfold layout:
row = ci*9 + a*3 + b for tap (a,b)), bias [1,128,1,1], stride 1, pad 1.
Output [32,128,64,64].

Strategy: data-parallel over batch — 4 images per NeuronCore, 8 cores.

Per image, implicit GEMM in bf16 (PSUM accumulates fp32; rel err ~2e-3
vs the 2e-2 gate).  The host pre-builds a [128, 66, 66] bf16 tile per
image with zero borders: partitions 0:64 hold the padded image P
(P[r] = img row r-1), partitions 64:128 hold the one-row-down copy U
(U[r] = img row r).  One contiguous HBM DMA per image, zero on-device
data prep.

Every 8-row output block is then uniform (no border special cases):
  out[co,y,x] = sum_{a,b,ci} W[ci,a,b,co] * img[ci, y+a-1, x+b-1]
  - taps (0,b)+(1,b): K=128 matmul, lhsT = [W[:,0,b,:]; W[:,1,b,:]],
    rhs = xim[:, y0:y0+8, b:b+64]           (3 matmuls)
  - tap (2,b): K=128 matmul with ZERO weights on partitions 0:64 and
    W[:,2,b,:] on 64:128, rhs = xim[:, y0+1:y0+9, b:b+64]  (3 matmuls)
    (K=64 matmuls would leave half the PE array idle, which keeps the
    HAM activity monitor from un-throttling the PE clock to 2.4 GHz —
    measured: any K=64 in the stream pins issue at 1.2 GHz.)

~26 dummy matmuls on a scratch SBUF tile run while the first image's
DMA is in flight, so the PE is already warm (K=8/8) when real work
starts.  PSUM eviction (fused bias add) alternates between ScalarE
(activation Identity + bias) and VectorE (tensor_scalar_add) so
neither engine gates the PE.  Output DMAs go in 32-row halves, issued
from ScalarE's hardware DGE so the Sync engine's serialized issue
stream only carries input DMAs.
"""
import sys

for _p in ("/opt/trn_rl_repo", "/root/.axon_site/_ro/trn_rl_repo"):
    if _p not in sys.path:
        sys.path.append(_p)

import numpy as np
import ml_dtypes
from contextlib import ExitStack

import concourse.bacc as bacc
import concourse.tile as tile
from concourse import mybir
from concourse.bass_utils import run_bass_kernel_spmd

f32 = mybir.dt.float32
bf16 = mybir.dt.bfloat16

N_CORES = 8
NB = 4  # images per core
N_WARM = 26  # dummy matmuls to warm the PE during the input DMA window


def build_nc():
    nc = bacc.Bacc()
    # host-built per-image tile: [P(66x66, zero borders) ; U(P shifted
    # one row up, row 65 = 0)], bf16
    xp = nc.declare_dram_parameter("x", [NB, 128, 66, 66], bf16, isOutput=False)
    # host-packed weights: [128, 6, 128] bf16
    #   w[:, b, :]   = [W[:,0,b,:]; W[:,1,b,:]]  (tap pair a=0,1)
    #   w[:, 3+b, :] = [0         ; W[:,2,b,:]]  (tap 2, zero-padded K=128)
    w = nc.declare_dram_parameter("w", [128, 6, 128], bf16, isOutput=False)
    bias = nc.declare_dram_parameter("b", [128, 1], f32, isOutput=False)
    out = nc.declare_dram_parameter("out", [NB, 128, 64, 64], f32, isOutput=True)

    with tile.TileContext(nc) as tc, ExitStack() as ctx:
        const = ctx.enter_context(tc.tile_pool(name="const", bufs=1))
        xim_pool = ctx.enter_context(tc.tile_pool(name="xim", bufs=3))
        ob_pool = ctx.enter_context(tc.tile_pool(name="ob", bufs=2))
        ps_pool = ctx.enter_context(tc.tile_pool(name="ps", bufs=7, space="PSUM"))
        pw_pool = ctx.enter_context(tc.tile_pool(name="pw", bufs=1, space="PSUM"))

        wt = const.tile([128, 6, 128], bf16)
        bt = const.tile([128, 1], f32)
        dummy = const.tile([128, 640], bf16)

        xims = []
        for n in range(NB):
            xim = xim_pool.tile([128, 66, 66], bf16)
            if n == 0:
                # priority prefix: rows 0:20 cover blocks 0-1, so real
                # matmuls start ~3us before the full image lands
                nc.sync.dma_start(out=xim[:, 0:20, :], in_=xp[n, :, 0:20, :])
                nc.sync.dma_start(out=xim[:, 20:66, :], in_=xp[n, :, 20:66, :])
            else:
                nc.sync.dma_start(out=xim[:, :, :], in_=xp[n])
            xims.append(xim)
            if n == 0:
                nc.sync.dma_start(out=wt[:], in_=w[:])
                nc.sync.dma_start(out=bt[:], in_=bias[:])

        # PE warm-up: independent matmuls into a scratch PSUM bank while
        # the input DMAs stream.  ~3.4us of sustained PE activity flips
        # the HAM clock gate to 2.4 GHz before the real stream begins.
        nc.gpsimd.memset(dummy[:], 0.0)
        Pw = pw_pool.tile([128, 8, 64], f32)
        for i in range(N_WARM):
            nc.tensor.matmul(
                Pw[:, :, :], dummy[:, 0:128],
                dummy[:, 128:640].rearrange("p (a b) -> p a b", a=8),
                start=True, stop=True,
            )

        for n in range(NB):
            xim = xims[n]
            osb = ob_pool.tile([128, 64, 64], f32)
            for blk in range(8):
                y0 = blk * 8
                P = ps_pool.tile([128, 8, 64], f32)
                for k, b in enumerate((0, 1, 2)):
                    nc.tensor.matmul(
                        P[:, :, :],
                        wt[:, b, :],
                        xim[:, y0:y0 + 8, b:b + 64],
                        start=(k == 0), stop=False,
                    )
                for k, b in enumerate((0, 1, 2)):
                    nc.tensor.matmul(
                        P[:, :, :],
                        wt[:, 3 + b, :],
                        xim[:, y0 + 1:y0 + 9, b:b + 64],
                        start=False, stop=(k == 2),
                    )
                # PSUM evict + bias, alternating engines per bank
                if blk % 2 == 0:
                    nc.scalar.activation(
                        osb[:, y0:y0 + 8, :], P[:, :, :],
                        mybir.ActivationFunctionType.Identity,
                        bias=bt[:, :],
                    )
                else:
                    nc.vector.tensor_scalar_add(
                        osb[:, y0:y0 + 8, :], P[:, :, :], bt[:, :])
                # drain the output in 16-row chunks so the final store
                # after the last matmul is tiny; issue from ScalarE's DGE
                if blk % 2 == 1:
                    r0 = (blk - 1) * 8
                    nc.scalar.dma_start(out=out[n, :, r0:r0 + 16, :],
                                        in_=osb[:, r0:r0 + 16, :])

    nc.finalize()
    return nc


_NC = None


def _get_nc():
    global _NC
    if _NC is None:
        _NC = build_nc()
    return _NC


def _prep(inputs):
    x = np.asarray(inputs["input"], dtype=np.float32)
    w = np.asarray(inputs["weight"], dtype=np.float32)
    b = np.asarray(inputs["bias"], dtype=np.float32).reshape(128, 1)

    nimg = x.shape[0]
    xb = x.astype(ml_dtypes.bfloat16)
    xf = np.zeros((nimg, 128, 66, 66), dtype=ml_dtypes.bfloat16)
    xf[:, 0:64, 1:65, 1:65] = xb          # P: rows 1..64 = img rows 0..63
    xf[:, 64:128, 0:64, 1:65] = xb        # U: rows 0..63 = img rows 0..63

    wr = w.reshape(64, 3, 3, 128)  # [ci, a, b, co]
    wa = np.zeros((128, 6, 128), dtype=ml_dtypes.bfloat16)
    for bb in range(3):
        wa[0:64, bb, :] = wr[:, 0, bb, :]
        wa[64:128, bb, :] = wr[:, 1, bb, :]
        wa[64:128, 3 + bb, :] = wr[:, 2, bb, :]
    return xf, wa, np.ascontiguousarray(b)


def kernel(**inputs) -> np.ndarray:
    xf, wa, b = _prep(inputs)
    nc = _get_nc()
    in_maps = [
        {"x": xf[c * NB:(c + 1) * NB], "w": wa, "b": b} for c in range(N_CORES)
    ]
    res = run_bass_kernel_spmd(nc, in_maps, list(range(N_CORES)))
    return np.concatenate([r["out"] for r in res.results], axis=0)
